# revision 11
# baseline (speedup 1.0000x reference)
"""Multi-head causal self-attention block for Trainium2, data-parallel over 8 cores.

Reference computation (per batch b of x [B=32, T=1024, C=384]):
    qkv = x @ W_attn;  q,k,v heads (H=6, D=64)
    y   = softmax(causal(q k^T / sqrt(D))) @ v
    out = y @ W_proj + b_proj

Sharding: batch dim 32 -> 4 per core, weights replicated, no collectives.

Per-core layout strategy (matmuls in float32r, moving dim >= 256 where it matters):
  - x^T [C, T] built via PE transposes (contraction must sit on partitions).
  - q^T,k^T per head-pair [128, T] tiles straight out of the QKV matmul
    (W_attn slices stationary); v kept natural [T, D] per head with a ones
    column appended so the attention row-sum (softmax denominator) drops out
    of the y-matmul for free.
  - scores computed transposed s^T[k, q] = k^T.T @ q^T; the two heads of a
    pair are row-tiled matmuls (K=64, base partitions 0/64) into one
    [128, 1024] psum pair tile; diagonal tiles only compute the valid
    column range.
  - exp on ACT reads psum with scale=1/8 folded in, writing both heads via
    one strided call; causal diagonal blocks fixed up by gpsimd
    affine_select, fully-masked prefixes zeroed by gpsimd memset.
  - y^T_aug[65, q] = v_aug.T @ p^T accumulated over k-tiles; row 64 is the
    softmax denominator. Normalization: evict psum once, reshape the
    denominator row to [128, 4] via an SBUF->SBUF DMA (wide lanes), DVE
    reciprocal, DRAM-bounce broadcast to [64, 512], one fused multiply.
  - out = proj(y^T) with bias added during psum eviction.
  - batches are software-pipelined: batch b's projection is emitted after
    batch b+1's prep so the PE instruction stream has independent work
    while the normalization DMA chain of batch b drains.
"""

import sys

if "/opt/trn_rl_repo" not in sys.path:
    sys.path.insert(0, "/opt/trn_rl_repo")

import numpy as np

B, T, C = 32, 1024, 384
H, D = 6, 64
NCORES = 8
BPC = B // NCORES          # batches per core
NPAIR = H // 2             # head pairs
TT = T // 128              # token tiles per batch (8)
QC = T // 512              # q chunks per batch (2)
VSTRIDE = H * (D + 1)      # 390: per-token-tile v_aug row width
PTW = (T // 128) * 512     # 4096: per-head pT block width

_nc_cache = {}


def _build_nc():
    import concourse.mybir as mybir
    from concourse import bacc
    from concourse.tile import TileContext
    from concourse.masks import make_identity

    f32 = mybir.dt.float32
    f32r = mybir.dt.float32r
    Exp = mybir.ActivationFunctionType.Exp
    GE = mybir.AluOpType.is_ge

    nc = bacc.Bacc("TRN2", target_bir_lowering=False, debug=False, num_devices=NCORES)

    x_d = nc.declare_dram_parameter("x", [BPC, T, C], f32, isOutput=False)
    wa_d = nc.declare_dram_parameter("W_attn", [C, 3 * C], f32, isOutput=False)
    wp_d = nc.declare_dram_parameter("W_proj", [C, C], f32, isOutput=False)
    bp_d = nc.declare_dram_parameter("b_proj", [C], f32, isOutput=False)
    out_d = nc.declare_dram_parameter("out", [BPC, T, C], f32, isOutput=True)

    dn_scratch = nc.dram_tensor("dn_scratch", [BPC, NPAIR, QC, 2, 512], f32)

    with TileContext(nc) as tc:
        with (
            tc.tile_pool(name="const", bufs=1) as const,
            tc.tile_pool(name="xin", bufs=10) as xin,
            tc.tile_pool(name="xT", bufs=2) as xTp,
            tc.tile_pool(name="qk", bufs=2) as qkp,
            tc.tile_pool(name="vb", bufs=2) as vbp,
            tc.tile_pool(name="pT", bufs=1) as pTp,
            tc.tile_pool(name="yh", bufs=2) as yhp,
            tc.tile_pool(name="Rb", bufs=3) as Rbp,
            tc.tile_pool(name="dn", bufs=4) as dnp,
            tc.tile_pool(name="yst", bufs=3) as ystp,
            tc.tile_pool(name="osb", bufs=2) as osbp,
            tc.tile_pool(name="psA", bufs=2, space="PSUM") as psA,   # 1 bank each
            tc.tile_pool(name="psS", bufs=2, space="PSUM") as psS,   # 2 banks each
            tc.tile_pool(name="psY", bufs=2, space="PSUM") as psY,   # 1 bank each
        ):
            identity = const.tile([128, 128], f32, tag="ident")
            make_identity(nc, identity[:])
            wa_sb = []
            for c in range(3):
                w = const.tile([128, 3 * C], f32r, tag=f"wa{c}")
                nc.gpsimd.dma_start(out=w[:], in_=wa_d[c * 128:(c + 1) * 128, :])
                wa_sb.append(w)
            wp_sb = []
            for c in range(3):
                w = const.tile([128, C], f32r, tag=f"wp{c}")
                nc.gpsimd.dma_start(out=w[:], in_=wp_d[c * 128:(c + 1) * 128, :])
                wp_sb.append(w)
            b_bc = const.tile([128, C], f32, tag="bbc")
            nc.sync.dma_start(
                out=b_bc[:], in_=bp_d[:].unsqueeze(0).broadcast_to([128, C])
            )

            def prep(b):
                """Load x, build x^T and v_aug for batch b."""
                xts = []
                for t in range(TT):
                    xt = xin.tile([128, C], f32, tag="xin", name=f"xt{b}_{t}")
                    nc.sync.dma_start(out=xt[:], in_=x_d[b, t * 128:(t + 1) * 128, :])
                    xts.append(xt)
                xT = xTp.tile([128, 3 * T], f32r, tag="xT", name=f"xT{b}")
                for c in range(3):
                    for half in range(2):
                        pst = psA.tile([128, 512], f32, tag="psA", name=f"pst{b}")
                        for j in range(4):
                            nc.tensor.transpose(
                                pst[:, j * 128:(j + 1) * 128],
                                xts[half * 4 + j][:, c * 128:(c + 1) * 128],
                                identity[:],
                            )
                        nc.scalar.copy(
                            xT[:, c * T + half * 512: c * T + half * 512 + 512],
                            pst[:],
                        )

                vb = vbp.tile([128, TT * VSTRIDE], f32r, tag="vb", name=f"vb{b}")
                nc.gpsimd.memset(
                    vb[:].bitcast(f32)
                    .rearrange("p (t h e) -> p t h e", t=TT, e=D + 1)[:, :, :, D:],
                    1.0,
                )
                for t in range(TT):
                    psv = psA.tile([128, 512], f32, tag="psA", name=f"psv{b}")
                    for c in range(3):
                        nc.tensor.matmul(
                            psv[:, 0:C],
                            lhsT=xT[:, c * T + t * 128: c * T + t * 128 + 128],
                            rhs=wa_sb[c][:, 2 * C: 3 * C],
                            start=(c == 0),
                            stop=(c == 2),
                        )
                    nc.scalar.activation(
                        vb[:, t * VSTRIDE: t * VSTRIDE + VSTRIDE]
                        .rearrange("p (h e) -> p h e", e=D + 1)[:, :, 0:D],
                        psv[:, 0:C].rearrange("p (h d) -> p h d", d=D),
                        mybir.ActivationFunctionType.Copy,
                    )
                return xT, vb

            def attn(b, xT, vb):
                """Attention for all head pairs of batch b; returns y^T buffer."""
                yh = yhp.tile([128, 3 * T], f32r, tag="yh", name=f"yh{b}")
                for p in range(NPAIR):
                    qk = qkp.tile([128, 2 * T], f32r, tag="qk", name=f"qk{b}_{p}")
                    for i, m in enumerate((p, 3 + p)):
                        for half in range(2):
                            psq = psA.tile([128, 512], f32, tag="psA", name=f"psq{b}")
                            for c in range(3):
                                nc.tensor.matmul(
                                    psq[:],
                                    lhsT=wa_sb[c][:, m * 128:(m + 1) * 128],
                                    rhs=xT[:, c * T + half * 512:
                                           c * T + half * 512 + 512],
                                    start=(c == 0),
                                    stop=(c == 2),
                                )
                            nc.scalar.copy(
                                qk[:, i * T + half * 512: i * T + half * 512 + 512],
                                psq[:],
                            )

                    for qc in range(QC):
                        nkt = 4 * (qc + 1)
                        pT = pTp.tile([128, 2 * nkt * 512], f32r, tag=f"pTq{qc}",
                                      name=f"pT{b}_{p}_{qc}")
                        ptw = nkt * 512
                        ys = [psY.tile([D + 1, 512], f32, tag="psY",
                                       name=f"ys{b}{p}{qc}{hh}") for hh in range(2)]
                        for kt in range(nkt):
                            diag = kt >= qc * 4
                            o = (kt - qc * 4) * 128 if diag else 0
                            pss = psS.tile([128, 1024], f32, tag="psS",
                                           name=f"pss{b}{p}")
                            for hh in range(2):
                                nc.tensor.matmul(
                                    pss[:, hh * 512 + o:(hh + 1) * 512],
                                    lhsT=qk[hh * 64:(hh + 1) * 64,
                                            T + kt * 128: T + kt * 128 + 128],
                                    rhs=qk[hh * 64:(hh + 1) * 64,
                                           qc * 512 + o: qc * 512 + 512],
                                    start=True,
                                    stop=True,
                                )
                            # exp into pT, both heads in one strided call
                            nc.scalar.activation(
                                pT[:].rearrange("p (h w) -> p h w", h=2)
                                [:, :, kt * 512 + o: (kt + 1) * 512],
                                pss[:].rearrange("p (h w) -> p h w", h=2)
                                [:, :, o:512],
                                Exp,
                                scale=0.125,
                            )
                            if diag and o > 0:
                                nc.gpsimd.memset(
                                    pT[:].bitcast(f32)
                                    .rearrange("p (h w) -> p h w", h=2)
                                    [:, :, kt * 512: kt * 512 + o],
                                    0.0,
                                )
                            if diag:
                                for hh in range(2):
                                    blk = pT[:, hh * ptw + kt * 512 + o:
                                             hh * ptw + kt * 512 + o + 128]
                                    nc.gpsimd.affine_select(
                                        out=blk,
                                        in_=blk,
                                        compare_op=GE,
                                        fill=0.0,
                                        base=0,
                                        pattern=[[1, 128]],
                                        channel_multiplier=-1,
                                    )
                            for hh in range(2):
                                h = 2 * p + hh
                                nc.tensor.matmul(
                                    ys[hh][:],
                                    lhsT=vb[:, kt * VSTRIDE + h * (D + 1):
                                            kt * VSTRIDE + (h + 1) * (D + 1)],
                                    rhs=pT[:, hh * ptw + kt * 512:
                                           hh * ptw + (kt + 1) * 512],
                                    start=(kt == 0),
                                    stop=(kt == nkt - 1),
                                )
                        # normalize: evict, wide reciprocal, broadcast, multiply
                        for hh in range(2):
                            col = p * T + qc * 512
                            yst = ystp.tile([D + 1, 512], f32, tag="yst",
                                            name=f"yst{b}{p}{qc}{hh}")
                            nc.vector.tensor_copy(yst[:], ys[hh][:])
                            dnr = dnp.tile([128, 4], f32, tag="dn",
                                           name=f"dnr{b}{p}{qc}{hh}")
                            nc.sync.dma_start(out=dnr[:], in_=yst[D:D + 1, :])
                            nc.vector.reciprocal(dnr[:], dnr[:])
                            nc.sync.dma_start(
                                out=dn_scratch[b, p, qc, hh, :]
                                .rearrange("(p f) -> p f", f=4),
                                in_=dnr[:],
                            )
                            Rb = Rbp.tile([D, 512], f32, tag="Rb",
                                          name=f"Rb{b}{p}{qc}{hh}")
                            nc.sync.dma_start(
                                out=Rb[:],
                                in_=dn_scratch[b, p, qc, hh, :]
                                .unsqueeze(0).broadcast_to([D, 512]),
                            )
                            nc.vector.tensor_mul(
                                yh[hh * 64:(hh + 1) * 64, col: col + 512],
                                yst[0:D, :],
                                Rb[:],
                            )
                return yh

            def proj(b, yh):
                for t in range(TT):
                    pso = psA.tile([128, 512], f32, tag="psA", name=f"pso{b}")
                    for c in range(3):
                        nc.tensor.matmul(
                            pso[:, 0:C],
                            lhsT=yh[:, c * T + t * 128: c * T + t * 128 + 128],
                            rhs=wp_sb[c][:],
                            start=(c == 0),
                            stop=(c == 2),
                        )
                    osb = osbp.tile([128, C], f32, tag="osb", name=f"osb{b}")
                    nc.vector.tensor_add(osb[:], pso[:, 0:C], b_bc[:])
                    nc.sync.dma_start(
                        out=out_d[b, t * 128:(t + 1) * 128, :], in_=osb[:]
                    )

            # software pipeline: proj(b-1) emitted after prep(b) so the PE
            # stream has fresh work while b-1's normalization DMAs drain
            xT0, vb0 = prep(0)
            yh_prev = attn(0, xT0, vb0)
            for b in range(1, BPC):
                xTb, vbb = prep(b)
                proj(b - 1, yh_prev)
                yh_prev = attn(b, xTb, vbb)
            proj(BPC - 1, yh_prev)

    nc.finalize()
    return nc


def _run(inputs, trace=False, **kw):
    from concourse.bass_utils import run_bass_kernel_spmd

    if "nc" not in _nc_cache:
        _nc_cache["nc"] = _build_nc()
    nc = _nc_cache["nc"]

    x = np.ascontiguousarray(np.asarray(inputs["x"], dtype=np.float32))
    wa = np.ascontiguousarray(np.asarray(inputs["W_attn"], dtype=np.float32))
    wp = np.ascontiguousarray(np.asarray(inputs["W_proj"], dtype=np.float32))
    bp = np.ascontiguousarray(np.asarray(inputs["b_proj"], dtype=np.float32))

    in_maps = [
        {"x": x[i * BPC:(i + 1) * BPC], "W_attn": wa, "W_proj": wp, "b_proj": bp}
        for i in range(NCORES)
    ]
    res = run_bass_kernel_spmd(nc, in_maps, list(range(NCORES)), trace=trace, **kw)
    out = np.concatenate([res.results[i]["out"] for i in range(NCORES)], axis=0)
    return out, res


def kernel(**inputs) -> np.ndarray:
    out, _ = _run(inputs, trace=False)
    return out


# revision 13
# speedup vs baseline: 1.0114x; 1.0114x over previous
"""Multi-head causal self-attention block for Trainium2, data-parallel over 8 cores.

Reference computation (per batch b of x [B=32, T=1024, C=384]):
    qkv = x @ W_attn;  q,k,v heads (H=6, D=64)
    y   = softmax(causal(q k^T / sqrt(D))) @ v
    out = y @ W_proj + b_proj

Sharding: batch dim 32 -> 4 per core, weights replicated, no collectives.

Per-core layout strategy (matmuls in float32r, moving dim >= 256 where it matters):
  - x^T [C, T] built via PE transposes (contraction must sit on partitions).
  - q^T,k^T per head-pair [128, T] tiles straight out of the QKV matmul
    (W_attn slices stationary); v kept natural [T, D] per head with a ones
    column appended so the attention row-sum (softmax denominator) drops out
    of the y-matmul for free.
  - scores computed transposed s^T[k, q] = k^T.T @ q^T; the two heads of a
    pair are row-tiled matmuls (K=64, base partitions 0/64) into one
    [128, 1024] psum pair tile; diagonal tiles only compute the valid
    column range.
  - exp on ACT reads psum with scale=1/8 folded in, writing both heads via
    one strided call; causal diagonal blocks fixed up by gpsimd
    affine_select, fully-masked prefixes zeroed by gpsimd memset.
  - y^T_aug[65, q] = v_aug.T @ p^T accumulated over k-tiles; row 64 is the
    softmax denominator. Normalization: evict psum once, reshape the
    denominator row to [128, 4] via an SBUF->SBUF DMA (wide lanes), DVE
    reciprocal, DRAM-bounce broadcast to [64, 512], one fused multiply.
  - out = proj(y^T) with bias added during psum eviction.
  - batches are software-pipelined: batch b's projection is emitted after
    batch b+1's prep so the PE instruction stream has independent work
    while the normalization DMA chain of batch b drains.
"""

import sys

if "/opt/trn_rl_repo" not in sys.path:
    sys.path.insert(0, "/opt/trn_rl_repo")

import numpy as np

B, T, C = 32, 1024, 384
H, D = 6, 64
NCORES = 8
BPC = B // NCORES          # batches per core
NPAIR = H // 2             # head pairs
TT = T // 128              # token tiles per batch (8)
QC = T // 512              # q chunks per batch (2)
VSTRIDE = H * (D + 1)      # 390: per-token-tile v_aug row width
PTW = (T // 128) * 512     # 4096: per-head pT block width

_nc_cache = {}


def _build_nc():
    import concourse.mybir as mybir
    from concourse import bacc
    from concourse.tile import TileContext
    from concourse.masks import make_identity

    f32 = mybir.dt.float32
    f32r = mybir.dt.float32r
    Exp = mybir.ActivationFunctionType.Exp
    GE = mybir.AluOpType.is_ge

    nc = bacc.Bacc("TRN2", target_bir_lowering=False, debug=False, num_devices=NCORES)

    x_d = nc.declare_dram_parameter("x", [BPC, T, C], f32, isOutput=False)
    wa_d = nc.declare_dram_parameter("W_attn", [C, 3 * C], f32, isOutput=False)
    wp_d = nc.declare_dram_parameter("W_proj", [C, C], f32, isOutput=False)
    bp_d = nc.declare_dram_parameter("b_proj", [C], f32, isOutput=False)
    out_d = nc.declare_dram_parameter("out", [BPC, T, C], f32, isOutput=True)

    dn_scratch = nc.dram_tensor("dn_scratch", [BPC, NPAIR, QC, 2, 512], f32)

    with TileContext(nc) as tc:
        with (
            tc.tile_pool(name="const", bufs=1) as const,
            tc.tile_pool(name="xin", bufs=10) as xin,
            tc.tile_pool(name="xT", bufs=2) as xTp,
            tc.tile_pool(name="qk", bufs=2) as qkp,
            tc.tile_pool(name="vb", bufs=2) as vbp,
            tc.tile_pool(name="pT", bufs=1) as pTp,
            tc.tile_pool(name="yh", bufs=2) as yhp,
            tc.tile_pool(name="Rb", bufs=3) as Rbp,
            tc.tile_pool(name="dn", bufs=4) as dnp,
            tc.tile_pool(name="yst", bufs=3) as ystp,
            tc.tile_pool(name="osb", bufs=2) as osbp,
            tc.tile_pool(name="psA", bufs=2, space="PSUM") as psA,   # 1 bank each
            tc.tile_pool(name="psS", bufs=2, space="PSUM") as psS,   # 2 banks each
            tc.tile_pool(name="psY", bufs=2, space="PSUM") as psY,   # 1 bank each
        ):
            identity = const.tile([128, 128], f32, tag="ident")
            make_identity(nc, identity[:])
            wa_sb = []
            wp_sb = []
            for c in range(3):
                wf = const.tile([128, 3 * C], f32, tag=f"waf{c}")
                nc.sync.dma_start(out=wf[:], in_=wa_d[c * 128:(c + 1) * 128, :])
                w = const.tile([128, 3 * C], f32r, tag=f"wa{c}")
                nc.vector.tensor_copy(w[:], wf[:])
                wa_sb.append(w)
                pf = const.tile([128, C], f32, tag=f"wpf{c}")
                nc.sync.dma_start(out=pf[:], in_=wp_d[c * 128:(c + 1) * 128, :])
                p = const.tile([128, C], f32r, tag=f"wp{c}")
                nc.vector.tensor_copy(p[:], pf[:])
                wp_sb.append(p)
            b_bc = const.tile([128, C], f32, tag="bbc")
            nc.sync.dma_start(
                out=b_bc[:], in_=bp_d[:].unsqueeze(0).broadcast_to([128, C])
            )

            def prep(b):
                """Load x, build x^T and v_aug for batch b."""
                xts = []
                for t in range(TT):
                    xt = xin.tile([128, C], f32, tag="xin", name=f"xt{b}_{t}")
                    nc.sync.dma_start(out=xt[:], in_=x_d[b, t * 128:(t + 1) * 128, :])
                    xts.append(xt)
                xT = xTp.tile([128, 3 * T], f32r, tag="xT", name=f"xT{b}")
                for c in range(3):
                    for half in range(2):
                        pst = psA.tile([128, 512], f32, tag="psA", name=f"pst{b}")
                        for j in range(4):
                            nc.tensor.transpose(
                                pst[:, j * 128:(j + 1) * 128],
                                xts[half * 4 + j][:, c * 128:(c + 1) * 128],
                                identity[:],
                            )
                        nc.vector.tensor_copy(
                            xT[:, c * T + half * 512: c * T + half * 512 + 512],
                            pst[:],
                        )

                vb = vbp.tile([128, TT * VSTRIDE], f32r, tag="vb", name=f"vb{b}")
                nc.gpsimd.memset(
                    vb[:].bitcast(f32)
                    .rearrange("p (t h e) -> p t h e", t=TT, e=D + 1)[:, :, :, D:],
                    1.0,
                )
                for t in range(TT):
                    psv = psA.tile([128, 512], f32, tag="psA", name=f"psv{b}")
                    for c in range(3):
                        nc.tensor.matmul(
                            psv[:, 0:C],
                            lhsT=xT[:, c * T + t * 128: c * T + t * 128 + 128],
                            rhs=wa_sb[c][:, 2 * C: 3 * C],
                            start=(c == 0),
                            stop=(c == 2),
                        )
                    nc.scalar.activation(
                        vb[:, t * VSTRIDE: t * VSTRIDE + VSTRIDE]
                        .rearrange("p (h e) -> p h e", e=D + 1)[:, :, 0:D],
                        psv[:, 0:C].rearrange("p (h d) -> p h d", d=D),
                        mybir.ActivationFunctionType.Copy,
                    )
                return xT, vb

            def attn(b, xT, vb, pending=None):
                """Attention for batch b; emits `pending` (previous batch's
                projection) after pair 0's qk production so its psum slot
                waits sit behind fast-releasing neighbors."""
                yh = yhp.tile([128, 3 * T], f32r, tag="yh", name=f"yh{b}")
                for p in range(NPAIR):
                    if p == 1 and pending is not None:
                        pending()
                    qk = qkp.tile([128, 2 * T], f32r, tag="qk", name=f"qk{b}_{p}")
                    for i, m in enumerate((p, 3 + p)):
                        for half in range(2):
                            psq = psA.tile([128, 512], f32, tag="psA", name=f"psq{b}")
                            for c in range(3):
                                nc.tensor.matmul(
                                    psq[:],
                                    lhsT=wa_sb[c][:, m * 128:(m + 1) * 128],
                                    rhs=xT[:, c * T + half * 512:
                                           c * T + half * 512 + 512],
                                    start=(c == 0),
                                    stop=(c == 2),
                                )
                            nc.vector.tensor_copy(
                                qk[:, i * T + half * 512: i * T + half * 512 + 512],
                                psq[:],
                            )

                    for qc in range(QC):
                        nkt = 4 * (qc + 1)
                        pT = pTp.tile([128, 2 * nkt * 512], f32r, tag=f"pTq{qc}",
                                      name=f"pT{b}_{p}_{qc}")
                        ptw = nkt * 512
                        ys = [psY.tile([D + 1, 512], f32, tag="psY",
                                       name=f"ys{b}{p}{qc}{hh}") for hh in range(2)]
                        for kt in range(nkt):
                            diag = kt >= qc * 4
                            o = (kt - qc * 4) * 128 if diag else 0
                            pss = psS.tile([128, 1024], f32, tag="psS",
                                           name=f"pss{b}{p}")
                            for hh in range(2):
                                nc.tensor.matmul(
                                    pss[:, hh * 512 + o:(hh + 1) * 512],
                                    lhsT=qk[hh * 64:(hh + 1) * 64,
                                            T + kt * 128: T + kt * 128 + 128],
                                    rhs=qk[hh * 64:(hh + 1) * 64,
                                           qc * 512 + o: qc * 512 + 512],
                                    start=True,
                                    stop=True,
                                )
                            # exp into pT, both heads in one strided call
                            nc.scalar.activation(
                                pT[:].rearrange("p (h w) -> p h w", h=2)
                                [:, :, kt * 512 + o: (kt + 1) * 512],
                                pss[:].rearrange("p (h w) -> p h w", h=2)
                                [:, :, o:512],
                                Exp,
                                scale=0.125,
                            )
                            if diag and o > 0:
                                nc.gpsimd.memset(
                                    pT[:].bitcast(f32)
                                    .rearrange("p (h w) -> p h w", h=2)
                                    [:, :, kt * 512: kt * 512 + o],
                                    0.0,
                                )
                            if diag:
                                for hh in range(2):
                                    blk = pT[:, hh * ptw + kt * 512 + o:
                                             hh * ptw + kt * 512 + o + 128]
                                    nc.gpsimd.affine_select(
                                        out=blk,
                                        in_=blk,
                                        compare_op=GE,
                                        fill=0.0,
                                        base=0,
                                        pattern=[[1, 128]],
                                        channel_multiplier=-1,
                                    )
                            for hh in range(2):
                                h = 2 * p + hh
                                nc.tensor.matmul(
                                    ys[hh][:],
                                    lhsT=vb[:, kt * VSTRIDE + h * (D + 1):
                                            kt * VSTRIDE + (h + 1) * (D + 1)],
                                    rhs=pT[:, hh * ptw + kt * 512:
                                           hh * ptw + (kt + 1) * 512],
                                    start=(kt == 0),
                                    stop=(kt == nkt - 1),
                                )
                        # normalize: evict, wide reciprocal, broadcast, multiply
                        for hh in range(2):
                            col = p * T + qc * 512
                            yst = ystp.tile([D + 1, 512], f32, tag="yst",
                                            name=f"yst{b}{p}{qc}{hh}")
                            nc.vector.tensor_copy(yst[:], ys[hh][:])
                            dnr = dnp.tile([128, 4], f32, tag="dn",
                                           name=f"dnr{b}{p}{qc}{hh}")
                            nc.sync.dma_start(out=dnr[:], in_=yst[D:D + 1, :])
                            nc.vector.reciprocal(dnr[:], dnr[:])
                            nc.sync.dma_start(
                                out=dn_scratch[b, p, qc, hh, :]
                                .rearrange("(p f) -> p f", f=4),
                                in_=dnr[:],
                            )
                            Rb = Rbp.tile([D, 512], f32, tag="Rb",
                                          name=f"Rb{b}{p}{qc}{hh}")
                            nc.sync.dma_start(
                                out=Rb[:],
                                in_=dn_scratch[b, p, qc, hh, :]
                                .unsqueeze(0).broadcast_to([D, 512]),
                            )
                            nc.vector.tensor_mul(
                                yh[hh * 64:(hh + 1) * 64, col: col + 512],
                                yst[0:D, :],
                                Rb[:],
                            )
                return yh

            def proj(b, yh):
                for t in range(TT):
                    pso = psA.tile([128, 512], f32, tag="psA", name=f"pso{b}")
                    for c in range(3):
                        nc.tensor.matmul(
                            pso[:, 0:C],
                            lhsT=yh[:, c * T + t * 128: c * T + t * 128 + 128],
                            rhs=wp_sb[c][:],
                            start=(c == 0),
                            stop=(c == 2),
                        )
                    osb = osbp.tile([128, C], f32, tag="osb", name=f"osb{b}")
                    nc.vector.tensor_add(osb[:], pso[:, 0:C], b_bc[:])
                    nc.sync.dma_start(
                        out=out_d[b, t * 128:(t + 1) * 128, :], in_=osb[:]
                    )

            # software pipeline: proj(b-1) emitted inside attn(b) (after its
            # first qk phase) so the PE stream has fresh work while b-1's
            # normalization DMAs drain
            import functools
            xT0, vb0 = prep(0)
            yh_prev = attn(0, xT0, vb0)
            for b in range(1, BPC):
                xTb, vbb = prep(b)
                yh_new = attn(b, xTb, vbb,
                              pending=functools.partial(proj, b - 1, yh_prev))
                yh_prev = yh_new
            proj(BPC - 1, yh_prev)

    nc.finalize()
    return nc


def _run(inputs, trace=False, **kw):
    from concourse.bass_utils import run_bass_kernel_spmd

    if "nc" not in _nc_cache:
        _nc_cache["nc"] = _build_nc()
    nc = _nc_cache["nc"]

    x = np.ascontiguousarray(np.asarray(inputs["x"], dtype=np.float32))
    wa = np.ascontiguousarray(np.asarray(inputs["W_attn"], dtype=np.float32))
    wp = np.ascontiguousarray(np.asarray(inputs["W_proj"], dtype=np.float32))
    bp = np.ascontiguousarray(np.asarray(inputs["b_proj"], dtype=np.float32))

    in_maps = [
        {"x": x[i * BPC:(i + 1) * BPC], "W_attn": wa, "W_proj": wp, "b_proj": bp}
        for i in range(NCORES)
    ]
    res = run_bass_kernel_spmd(nc, in_maps, list(range(NCORES)), trace=trace, **kw)
    out = np.concatenate([res.results[i]["out"] for i in range(NCORES)], axis=0)
    return out, res


def kernel(**inputs) -> np.ndarray:
    out, _ = _run(inputs, trace=False)
    return out


# revision 14
# speedup vs baseline: 1.0626x; 1.0505x over previous
"""Multi-head causal self-attention block for Trainium2, data-parallel over 8 cores.

Reference computation (per batch b of x [B=32, T=1024, C=384]):
    qkv = x @ W_attn;  q,k,v heads (H=6, D=64)
    y   = softmax(causal(q k^T / sqrt(D))) @ v
    out = y @ W_proj + b_proj

Sharding: batch dim 32 -> 4 per core, weights replicated, no collectives.

Per-core layout strategy (matmuls in float32r, moving dim >= 256 where it matters):
  - x^T [C, T] built via PE transposes (contraction must sit on partitions).
  - q^T,k^T per head-pair [128, T] tiles straight out of the QKV matmul
    (W_attn slices stationary); v kept natural [T, D] per head with a ones
    column appended so the attention row-sum (softmax denominator) drops out
    of the y-matmul for free.
  - scores computed transposed s^T[k, q] = k^T.T @ q^T; the two heads of a
    pair are row-tiled matmuls (K=64, base partitions 0/64) into one
    [128, 1024] psum pair tile; diagonal tiles only compute the valid
    column range.
  - exp on ACT reads psum with scale=1/8 folded in, writing both heads via
    one strided call; causal diagonal blocks fixed up by gpsimd
    affine_select, fully-masked prefixes zeroed by gpsimd memset.
  - y^T_aug[65, q] = v_aug.T @ p^T accumulated over k-tiles; row 64 is the
    softmax denominator. Normalization: evict psum once, reshape the
    denominator row to [128, 4] via an SBUF->SBUF DMA (wide lanes), DVE
    reciprocal, DRAM-bounce broadcast to [64, 512], one fused multiply.
  - out = proj(y^T) with bias added during psum eviction.
  - batches are software-pipelined: batch b's projection is emitted after
    batch b+1's prep so the PE instruction stream has independent work
    while the normalization DMA chain of batch b drains.
"""

import sys

if "/opt/trn_rl_repo" not in sys.path:
    sys.path.insert(0, "/opt/trn_rl_repo")

import numpy as np

B, T, C = 32, 1024, 384
H, D = 6, 64
NCORES = 8
BPC = B // NCORES          # batches per core
NPAIR = H // 2             # head pairs
TT = T // 128              # token tiles per batch (8)
QC = T // 512              # q chunks per batch (2)
VSTRIDE = H * (D + 1)      # 390: per-token-tile v_aug row width
PTW = (T // 128) * 512     # 4096: per-head pT block width

_nc_cache = {}


def _build_nc():
    import concourse.mybir as mybir
    from concourse import bacc
    from concourse.tile import TileContext
    from concourse.masks import make_identity

    f32 = mybir.dt.float32
    f32r = mybir.dt.float32r
    Exp = mybir.ActivationFunctionType.Exp
    GE = mybir.AluOpType.is_ge

    nc = bacc.Bacc("TRN2", target_bir_lowering=False, debug=False, num_devices=NCORES)

    x_d = nc.declare_dram_parameter("x", [BPC, T, C], f32, isOutput=False)
    wa_d = nc.declare_dram_parameter("W_attn", [C, 3 * C], f32, isOutput=False)
    wp_d = nc.declare_dram_parameter("W_proj", [C, C], f32, isOutput=False)
    bp_d = nc.declare_dram_parameter("b_proj", [C], f32, isOutput=False)
    out_d = nc.declare_dram_parameter("out", [BPC, T, C], f32, isOutput=True)

    dn_scratch = nc.dram_tensor("dn_scratch", [BPC, NPAIR, QC, 2, 512], f32)

    with TileContext(nc) as tc:
        with (
            tc.tile_pool(name="const", bufs=1) as const,
            tc.tile_pool(name="xin", bufs=10) as xin,
            tc.tile_pool(name="xT", bufs=2) as xTp,
            tc.tile_pool(name="qk", bufs=2) as qkp,
            tc.tile_pool(name="vb", bufs=2) as vbp,
            tc.tile_pool(name="pT", bufs=1) as pTp,
            tc.tile_pool(name="yh", bufs=2) as yhp,
            tc.tile_pool(name="Rb", bufs=3) as Rbp,
            tc.tile_pool(name="dn", bufs=4) as dnp,
            tc.tile_pool(name="yst", bufs=3) as ystp,
            tc.tile_pool(name="osb", bufs=2) as osbp,
            tc.tile_pool(name="psA", bufs=2, space="PSUM") as psA,   # 1 bank each
            tc.tile_pool(name="psS", bufs=2, space="PSUM") as psS,   # 2 banks each
            tc.tile_pool(name="psY", bufs=2, space="PSUM") as psY,   # 1 bank each
        ):
            identity = const.tile([128, 128], f32, tag="ident")
            make_identity(nc, identity[:])
            wa_sb = []
            wp_sb = []
            for c in range(3):
                wf = const.tile([128, 3 * C], f32, tag=f"waf{c}")
                nc.sync.dma_start(out=wf[:], in_=wa_d[c * 128:(c + 1) * 128, :])
                w = const.tile([128, 3 * C], f32r, tag=f"wa{c}")
                nc.vector.tensor_copy(w[:], wf[:])
                wa_sb.append(w)
                pf = const.tile([128, C], f32, tag=f"wpf{c}")
                nc.sync.dma_start(out=pf[:], in_=wp_d[c * 128:(c + 1) * 128, :])
                p = const.tile([128, C], f32r, tag=f"wp{c}")
                nc.vector.tensor_copy(p[:], pf[:])
                wp_sb.append(p)
            b_bc = const.tile([128, C], f32, tag="bbc")
            nc.sync.dma_start(
                out=b_bc[:], in_=bp_d[:].unsqueeze(0).broadcast_to([128, C])
            )

            def prep(b):
                """Load x, build x^T and v_aug for batch b."""
                xts = []
                for t in range(TT):
                    xt = xin.tile([128, C], f32, tag="xin", name=f"xt{b}_{t}")
                    nc.sync.dma_start(out=xt[:], in_=x_d[b, t * 128:(t + 1) * 128, :])
                    xts.append(xt)
                xT = xTp.tile([128, 3 * T], f32r, tag="xT", name=f"xT{b}")
                for c in range(3):
                    for half in range(2):
                        pst = psA.tile([128, 512], f32, tag="psA", name=f"pst{b}")
                        for j in range(4):
                            nc.tensor.transpose(
                                pst[:, j * 128:(j + 1) * 128],
                                xts[half * 4 + j][:, c * 128:(c + 1) * 128],
                                identity[:],
                            )
                        nc.vector.tensor_copy(
                            xT[:, c * T + half * 512: c * T + half * 512 + 512],
                            pst[:],
                        )

                vb = vbp.tile([128, TT * VSTRIDE], f32r, tag="vb", name=f"vb{b}")
                nc.gpsimd.memset(
                    vb[:].bitcast(f32)
                    .rearrange("p (t h e) -> p t h e", t=TT, e=D + 1)[:, :, :, D:],
                    1.0,
                )
                for t in range(TT):
                    psv = psA.tile([128, 512], f32, tag="psA", name=f"psv{b}")
                    for c in range(3):
                        nc.tensor.matmul(
                            psv[:, 0:C],
                            lhsT=xT[:, c * T + t * 128: c * T + t * 128 + 128],
                            rhs=wa_sb[c][:, 2 * C: 3 * C],
                            start=(c == 0),
                            stop=(c == 2),
                        )
                    nc.scalar.activation(
                        vb[:, t * VSTRIDE: t * VSTRIDE + VSTRIDE]
                        .rearrange("p (h e) -> p h e", e=D + 1)[:, :, 0:D],
                        psv[:, 0:C].rearrange("p (h d) -> p h d", d=D),
                        mybir.ActivationFunctionType.Copy,
                    )
                return xT, vb

            def attn(b, xT, vb, pending=None):
                """Attention for batch b; emits `pending` (previous batch's
                projection) after pair 0's qk production so its psum slot
                waits sit behind fast-releasing neighbors."""
                yh = yhp.tile([128, 3 * T], f32r, tag="yh", name=f"yh{b}")
                for p in range(NPAIR):
                    qk = qkp.tile([128, 2 * T], f32r, tag="qk", name=f"qk{b}_{p}")
                    for i, m in enumerate((p, 3 + p)):
                        for half in range(2):
                            psq = psA.tile([128, 512], f32, tag="psA", name=f"psq{b}")
                            for c in range(3):
                                nc.tensor.matmul(
                                    psq[:],
                                    lhsT=wa_sb[c][:, m * 128:(m + 1) * 128],
                                    rhs=xT[:, c * T + half * 512:
                                           c * T + half * 512 + 512],
                                    start=(c == 0),
                                    stop=(c == 2),
                                )
                            nc.vector.tensor_copy(
                                qk[:, i * T + half * 512: i * T + half * 512 + 512],
                                psq[:],
                            )

                    for qc in range(QC):
                        nkt = 4 * (qc + 1)
                        pT = pTp.tile([128, 2 * nkt * 512], f32r, tag=f"pTq{qc}",
                                      name=f"pT{b}_{p}_{qc}")
                        ptw = nkt * 512
                        ys = [psY.tile([D + 1, 512], f32, tag="psY",
                                       name=f"ys{b}{p}{qc}{hh}") for hh in range(2)]
                        for kt in range(nkt):
                            diag = kt >= qc * 4
                            o = (kt - qc * 4) * 128 if diag else 0
                            pss = psS.tile([128, 1024], f32, tag="psS",
                                           name=f"pss{b}{p}")
                            for hh in range(2):
                                nc.tensor.matmul(
                                    pss[:, hh * 512 + o:(hh + 1) * 512],
                                    lhsT=qk[hh * 64:(hh + 1) * 64,
                                            T + kt * 128: T + kt * 128 + 128],
                                    rhs=qk[hh * 64:(hh + 1) * 64,
                                           qc * 512 + o: qc * 512 + 512],
                                    start=True,
                                    stop=True,
                                )
                            # exp into pT, both heads in one strided call
                            nc.scalar.activation(
                                pT[:].rearrange("p (h w) -> p h w", h=2)
                                [:, :, kt * 512 + o: (kt + 1) * 512],
                                pss[:].rearrange("p (h w) -> p h w", h=2)
                                [:, :, o:512],
                                Exp,
                                scale=0.125,
                            )
                            if diag and o > 0:
                                nc.gpsimd.memset(
                                    pT[:].bitcast(f32)
                                    .rearrange("p (h w) -> p h w", h=2)
                                    [:, :, kt * 512: kt * 512 + o],
                                    0.0,
                                )
                            if diag:
                                for hh in range(2):
                                    blk = pT[:, hh * ptw + kt * 512 + o:
                                             hh * ptw + kt * 512 + o + 128]
                                    nc.gpsimd.affine_select(
                                        out=blk,
                                        in_=blk,
                                        compare_op=GE,
                                        fill=0.0,
                                        base=0,
                                        pattern=[[1, 128]],
                                        channel_multiplier=-1,
                                    )
                            for hh in range(2):
                                h = 2 * p + hh
                                nc.tensor.matmul(
                                    ys[hh][:],
                                    lhsT=vb[:, kt * VSTRIDE + h * (D + 1):
                                            kt * VSTRIDE + (h + 1) * (D + 1)],
                                    rhs=pT[:, hh * ptw + kt * 512:
                                           hh * ptw + (kt + 1) * 512],
                                    start=(kt == 0),
                                    stop=(kt == nkt - 1),
                                )
                        # normalize: evict, wide reciprocal, broadcast, multiply
                        for hh in range(2):
                            col = p * T + qc * 512
                            yst = ystp.tile([D + 1, 512], f32, tag="yst",
                                            name=f"yst{b}{p}{qc}{hh}")
                            nc.vector.tensor_copy(yst[:], ys[hh][:])
                            dnr = dnp.tile([128, 4], f32, tag="dn",
                                           name=f"dnr{b}{p}{qc}{hh}")
                            nc.sync.dma_start(out=dnr[:], in_=yst[D:D + 1, :])
                            nc.vector.reciprocal(dnr[:], dnr[:])
                            nc.sync.dma_start(
                                out=dn_scratch[b, p, qc, hh, :]
                                .rearrange("(p f) -> p f", f=4),
                                in_=dnr[:],
                            )
                            Rb = Rbp.tile([D, 512], f32, tag="Rb",
                                          name=f"Rb{b}{p}{qc}{hh}")
                            nc.sync.dma_start(
                                out=Rb[:],
                                in_=dn_scratch[b, p, qc, hh, :]
                                .unsqueeze(0).broadcast_to([D, 512]),
                            )
                            nc.vector.tensor_mul(
                                yh[hh * 64:(hh + 1) * 64, col: col + 512],
                                yst[0:D, :],
                                Rb[:],
                            )
                return yh

            def proj(b, yh):
                for t in range(TT):
                    pso = psA.tile([128, 512], f32, tag="psA", name=f"pso{b}")
                    for c in range(3):
                        nc.tensor.matmul(
                            pso[:, 0:C],
                            lhsT=yh[:, c * T + t * 128: c * T + t * 128 + 128],
                            rhs=wp_sb[c][:],
                            start=(c == 0),
                            stop=(c == 2),
                        )
                    osb = osbp.tile([128, C], f32, tag="osb", name=f"osb{b}")
                    nc.vector.tensor_add(osb[:], pso[:, 0:C], b_bc[:])
                    nc.sync.dma_start(
                        out=out_d[b, t * 128:(t + 1) * 128, :], in_=osb[:]
                    )

            # software pipeline: proj(b-1) emitted inside attn(b) (after its
            # first qk phase) so the PE stream has fresh work while b-1's
            # normalization DMAs drain
            import functools
            xT0, vb0 = prep(0)
            yh_prev = attn(0, xT0, vb0)
            for b in range(1, BPC):
                xTb, vbb = prep(b)
                proj(b - 1, yh_prev)
                yh_prev = attn(b, xTb, vbb)
            proj(BPC - 1, yh_prev)

    nc.finalize()
    return nc


def _run(inputs, trace=False, **kw):
    from concourse.bass_utils import run_bass_kernel_spmd

    if "nc" not in _nc_cache:
        _nc_cache["nc"] = _build_nc()
    nc = _nc_cache["nc"]

    x = np.ascontiguousarray(np.asarray(inputs["x"], dtype=np.float32))
    wa = np.ascontiguousarray(np.asarray(inputs["W_attn"], dtype=np.float32))
    wp = np.ascontiguousarray(np.asarray(inputs["W_proj"], dtype=np.float32))
    bp = np.ascontiguousarray(np.asarray(inputs["b_proj"], dtype=np.float32))

    in_maps = [
        {"x": x[i * BPC:(i + 1) * BPC], "W_attn": wa, "W_proj": wp, "b_proj": bp}
        for i in range(NCORES)
    ]
    res = run_bass_kernel_spmd(nc, in_maps, list(range(NCORES)), trace=trace, **kw)
    out = np.concatenate([res.results[i]["out"] for i in range(NCORES)], axis=0)
    return out, res


def kernel(**inputs) -> np.ndarray:
    out, _ = _run(inputs, trace=False)
    return out
